# revision 1
# baseline (speedup 1.0000x reference)
"""BNN-KDE ELBO kernel for Trainium2, data-parallel over the 8192 samples on 8 cores.

Math (matches the jax reference):
  out = data_lp - kl_term
  data_lp = mean_n sum_b [ -0.5*B*(y_pred[n,b]-y[b])^2 + 0.5*(log B - log 2pi) ]
  kl_term = mean_n [ logsumexp_k comp_lp[n,k] - log K - prior_lp[n] ]
  comp_lp[n,k] = -0.5*(D*log2pi + D*log var[k] + ||w_n - e_k||^2 / var[k])

Device work per core (1024 samples):
  - comp_lp via one PE matmul with contract dim 15:
      lhsT = [w (13 rows); ||w||^2; 1],  rhs = [e/var (13); -0.5/var; colk]
  - exp(comp_lp - m[n]) on ACT with per-partition bias and fused row-sum.
    m[n] = comp_lp[n, rand_idxs[n]] (host-computed; a valid logsumexp shift
    since it is <= the true row max and within 0.5*||eps_n||^2 of it).
  - tiny MLP y_pred: ACT tanh with per-partition scale/bias + fused DVE ops;
    sum_b (y_pred-y)^2 recovered from scalar_tensor_tensor / affine_mul_reduce
    accumulators on host.
Host: O(N*D) prep (gather, transposes), final scalar combine of per-core sums.
"""

import os
import sys

import numpy as np
import ml_dtypes
ml_bf16 = ml_dtypes.bfloat16

for _p in ("/opt/trn_rl_repo",):
    if _p not in sys.path and os.path.isdir(_p):
        sys.path.insert(0, _p)

NUM_NODES = 2
ALPHA = 1.0
BETA = 5.0
KL_BETA = 1.0
LOG_2PI = float(np.log(2.0 * np.pi))

K_COMP = 8192
N_SAMP = 8192
B_X = 2048
D_W = 13

N_CORES = 8
N_LOC = N_SAMP // N_CORES          # 1024 samples per core
P = 128                             # partitions
TILES = N_LOC // P                  # 8 sample-tiles per core
KCHUNK = 2048                       # psum-resident comp_lp chunk (4 banks)
NCHUNK = K_COMP // KCHUNK           # 4 chunks per sample-tile
KSUB = 512                          # fp32 matmul free-dim limit

# pcol column indices (13 weight cols as in reference layout, then -m)
_C_W10, _C_W11, _C_B10, _C_B11 = 0, 1, 2, 3
_C_W200, _C_W201, _C_W210, _C_W211 = 4, 5, 6, 7
_C_B20, _C_B21, _C_W30, _C_W31, _C_B3 = 8, 9, 10, 11, 12
_C_NEGM = 13
PCOLS = 14

_PROG = None
LAST_EXEC_NS = None


def build_program():
    import concourse.bass as bass
    import concourse.tile as tile
    from concourse import bacc, mybir

    f32 = mybir.dt.float32
    f32r = mybir.dt.float32r
    bf16 = mybir.dt.bfloat16
    Alu = mybir.AluOpType
    Act = mybir.ActivationFunctionType

    nc = bacc.Bacc("TRN2", target_bir_lowering=False, debug=False,
                   num_devices=N_CORES)

    empT_d = nc.declare_dram_parameter("empT", [15, K_COMP], f32r, isOutput=False)
    wT_d = nc.declare_dram_parameter("wT", [15, N_LOC], f32r, isOutput=False)
    pcol_d = nc.declare_dram_parameter("pcol", [N_LOC, PCOLS], f32, isOutput=False)
    xv_d = nc.declare_dram_parameter("xv", [B_X], bf16, isOutput=False)
    nyv_d = nc.declare_dram_parameter("nyv", [B_X], f32, isOutput=False)
    qparts_d = nc.declare_dram_parameter("qparts", [P, TILES * NCHUNK + KCHUNK // KSUB - 1], f32, isOutput=True)
    sv2_d = nc.declare_dram_parameter("sv2", [P, TILES], f32, isOutput=True)
    samr_d = nc.declare_dram_parameter("samr", [P, TILES], f32, isOutput=True)

    with tile.TileContext(nc) as tc:
        with (
            tc.tile_pool(name="const", bufs=1) as cpool,
            tc.tile_pool(name="hpool", bufs=3) as hpool,
            tc.tile_pool(name="h2pool", bufs=4) as h2pool,
            tc.tile_pool(name="vpool", bufs=2) as wpool,
            tc.tile_pool(name="spool", bufs=2) as spool,
            tc.tile_pool(name="dump", bufs=1) as dpool,
            tc.tile_pool(name="psum", bufs=2, space=bass.MemorySpace.PSUM) as ppool,
        ):
            empT = cpool.tile([15, K_COMP], f32r)
            nc.sync.dma_start(empT[:], empT_d[:])
            wT = cpool.tile([15, N_LOC], f32r)
            nc.sync.dma_start(wT[:], wT_d[:])
            warm = cpool.tile([P, 1], f32)
            nc.vector.memset(warm[:], 0.0)
            nc.scalar.activation(warm[:], warm[:], Act.Exp)

            pcs = []
            for t in range(TILES):
                pc = cpool.tile([P, PCOLS], f32, tag=f"pc{t}")
                nc.sync.dma_start(pc[:], pcol_d[t * P:(t + 1) * P, :])
                pcs.append(pc)

            xb = cpool.tile([P, B_X], bf16)
            nc.sync.dma_start(xb[:], xv_d[:].partition_broadcast(P))
            nyb = cpool.tile([P, B_X], f32)
            nc.sync.dma_start(nyb[:], nyv_d[:].partition_broadcast(P))

            qparts_sb = cpool.tile([P, TILES * NCHUNK + KCHUNK // KSUB - 1], f32)
            sv2_sb = cpool.tile([P, TILES], f32)
            samr_sb = cpool.tile([P, TILES], f32)

            def emit_mlp(t):
                pc = pcs[t]
                # ---- MLP block ----
                arg01 = spool.tile([P, 2 * B_X], bf16, tag="arg01")
                nc.vector.tensor_scalar(arg01[:, :B_X], xb[:],
                                        pc[:, _C_W10:_C_W10 + 1],
                                        pc[:, _C_B10:_C_B10 + 1],
                                        Alu.mult, Alu.add)
                nc.vector.tensor_scalar(arg01[:, B_X:], xb[:],
                                        pc[:, _C_W11:_C_W11 + 1],
                                        pc[:, _C_B11:_C_B11 + 1],
                                        Alu.mult, Alu.add)
                h01 = hpool.tile([P, 2 * B_X], bf16, tag="h01")
                nc.scalar.activation(h01[:], arg01[:], Act.Tanh)
                h0 = h01[:, :B_X]
                h1 = h01[:, B_X:]

                t0 = spool.tile([P, B_X], bf16, tag="t01")
                nc.vector.tensor_scalar(t0[:], h1,
                                        pc[:, _C_W201:_C_W201 + 1],
                                        pc[:, _C_B20:_C_B20 + 1],
                                        Alu.mult, Alu.add)
                p0 = spool.tile([P, B_X], bf16, tag="p01")
                nc.vector.tensor_scalar(p0[:], h0,
                                        pc[:, _C_W200:_C_W200 + 1], None,
                                        Alu.mult)
                r01 = spool.tile([P, 2 * B_X], bf16, tag="r01")
                nc.vector.tensor_tensor(r01[:, :B_X], p0[:], t0[:], Alu.add)
                t1 = spool.tile([P, B_X], bf16, tag="t01")
                nc.vector.tensor_scalar(t1[:], h1,
                                        pc[:, _C_W211:_C_W211 + 1],
                                        pc[:, _C_B21:_C_B21 + 1],
                                        Alu.mult, Alu.add)
                p1 = spool.tile([P, B_X], bf16, tag="p01")
                nc.vector.tensor_scalar(p1[:], h0,
                                        pc[:, _C_W210:_C_W210 + 1], None,
                                        Alu.mult)
                nc.vector.tensor_tensor(r01[:, B_X:], p1[:], t1[:], Alu.add)
                h2 = h2pool.tile([P, 2 * B_X], bf16, tag="h2")
                nc.scalar.activation(h2[:], r01[:], Act.Tanh)

                # v = w3_1*h2_1 - y ; v2 = w3_0*h2_0 + v = y_pred - y - b3
                v = spool.tile([P, B_X], f32, tag="v")
                nc.vector.scalar_tensor_tensor(v[:], h2[:, B_X:],
                                               pc[:, _C_W31:_C_W31 + 1],
                                               nyb[:], Alu.mult, Alu.add)
                v2 = wpool.tile([P, B_X], f32, tag="v2")
                nc.vector.scalar_tensor_tensor(v2[:], h2[:, :B_X],
                                               pc[:, _C_W30:_C_W30 + 1],
                                               v[:], Alu.mult, Alu.add,
                                               accum_out=sv2_sb[:, t:t + 1])
                # samr = sum (v2 + b3) * v2
                zdump = dpool.tile([P, B_X], f32, tag="zdump")
                nc.vector.affine_mul_reduce(zdump[:], samr_sb[:, t:t + 1],
                                            v2[:], v2[:],
                                            scale=1.0,
                                            bias=pc[:, _C_B3:_C_B3 + 1])

            def emit_kde(t):
                pc = pcs[t]
                lhsT = wT[:, t * P:(t + 1) * P]
                # ---- KDE block: comp_lp -> exp(. - m) -> row sums ----
                # Tile 0 chunk 0 runs exp per 512-wide matmul so ACT starts
                # ~3us earlier instead of waiting on 4 cold serial matmuls.
                for c in range(NCHUNK):
                    ps = ppool.tile([P, KCHUNK], f32, tag="ps")
                    sub = (t == 0 and c == 0)
                    for s in range(KCHUNK // KSUB):
                        k0 = c * KCHUNK + s * KSUB
                        nc.tensor.matmul(
                            ps[:, s * KSUB:(s + 1) * KSUB],
                            lhsT,
                            empT[:, k0:k0 + KSUB],
                            start=True, stop=True,
                        )
                        if sub:
                            qcol = 0 if s == 0 else TILES * NCHUNK + s - 1
                            nc.scalar.activation(
                                ps[:, s * KSUB:(s + 1) * KSUB],
                                ps[:, s * KSUB:(s + 1) * KSUB], Act.Exp,
                                bias=pc[:, _C_NEGM:_C_NEGM + 1], scale=1.0,
                                accum_out=qparts_sb[:, qcol:qcol + 1],
                            )
                    if not sub:
                        nc.scalar.activation(
                            ps[:], ps[:], Act.Exp,
                            bias=pc[:, _C_NEGM:_C_NEGM + 1], scale=1.0,
                            accum_out=qparts_sb[:, t * NCHUNK + c:t * NCHUNK + c + 1],
                        )

            # Tile-0 KDE first (its inputs land earliest: no broadcast-DMA
            # dependency), then every MLP block, then the remaining KDE
            # blocks: the trailing ~60us of ACT exp work has no DVE
            # dependents, so the DVE tail fully overlaps, and the scheduler
            # backfills any ACT idle slots with ready exp chunks.
            emit_kde(0)
            for t in range(TILES):
                emit_mlp(t)
            for t in range(1, TILES):
                emit_kde(t)

            nc.sync.dma_start(qparts_d[:], qparts_sb[:])
            nc.sync.dma_start(sv2_d[:], sv2_sb[:])
            nc.sync.dma_start(samr_d[:], samr_sb[:])

    nc.compile()
    return nc


def _get_prog():
    global _PROG
    if _PROG is None:
        _PROG = build_program()
    return _PROG


def host_prep(emp_samples, log_kde_rhos, x, y, eps, rand_idxs):
    """Returns (per-core in_maps, host-side combine context)."""
    emp = np.asarray(emp_samples, np.float32)
    logr = np.asarray(log_kde_rhos, np.float32)
    x = np.asarray(x, np.float32).reshape(-1)
    y = np.asarray(y, np.float32).reshape(-1)
    eps = np.asarray(eps, np.float32)
    idx = np.asarray(rand_idxs).astype(np.int64)

    # softplus in f32, matching jax.nn.softplus
    kde_std = np.logaddexp(np.float32(0.0), logr).astype(np.float32)
    kde_var = (kde_std * kde_std).astype(np.float32)

    esq = np.einsum("kd,kd->k", emp, emp, dtype=np.float32).astype(np.float32)
    colconst = (-0.5 * (D_W * LOG_2PI + D_W * np.log(kde_var))).astype(np.float32)
    a = (-0.5 / kde_var).astype(np.float32)

    # empT rows: e/var (13), a, colconst + a*esq
    empT = np.empty((15, K_COMP), np.float32)
    empT[:D_W] = (emp / kde_var[:, None]).T
    empT[D_W] = a
    empT[D_W + 1] = colconst + a * esq

    # per-sample things
    std_g = kde_std[idx]
    w = (emp[idx] + eps * std_g[:, None]).astype(np.float32)
    wsq = np.einsum("nd,nd->n", w, w, dtype=np.float32).astype(np.float32)
    epssq = np.einsum("nd,nd->n", eps, eps, dtype=np.float32)
    m = (colconst[idx] - 0.5 * epssq).astype(np.float32)

    in_maps = []
    for c in range(N_CORES):
        sl = slice(c * N_LOC, (c + 1) * N_LOC)
        wT = np.empty((15, N_LOC), np.float32)
        wT[:D_W] = w[sl].T
        wT[D_W] = wsq[sl]
        wT[D_W + 1] = 1.0
        pcol = np.empty((N_LOC, PCOLS), np.float32)
        pcol[:, :D_W] = w[sl]
        pcol[:, _C_NEGM] = -m[sl]
        in_maps.append({
            "empT": np.ascontiguousarray(empT),
            "wT": np.ascontiguousarray(wT),
            "pcol": np.ascontiguousarray(pcol),
            "xv": x.astype(ml_bf16),
            "nyv": np.ascontiguousarray(-y),
        })

    ctx = {"w": w, "wsq": wsq, "m": m, "b3": w[:, _C_B3], "y": y}
    return in_maps, ctx


def host_combine(ctx, qsum, sv2, samr):
    """qsum/sv2/samr are full [N_SAMP] float64 arrays gathered from cores."""
    m = ctx["m"].astype(np.float64)
    wsq = ctx["wsq"].astype(np.float64)
    b3 = ctx["b3"].astype(np.float64)
    y = ctx["y"].astype(np.float64)

    q_lp = m + np.log(qsum) - np.log(float(K_COMP))
    prior_lp = -0.5 * ALPHA * wsq + D_W * 0.5 * (np.log(ALPHA) - LOG_2PI)
    kl_term = np.mean(q_lp - prior_lp)

    ssq = samr + b3 * sv2 + B_X * b3 * b3   # sum_b (y_pred - y)^2 per sample
    data_lp = (-0.5 * BETA) * np.mean(ssq) + B_X * 0.5 * (np.log(BETA) - LOG_2PI)
    return np.float32(data_lp - KL_BETA * kl_term)


def kernel(emp_samples, log_kde_rhos, x, y, eps, rand_idxs):
    global LAST_EXEC_NS
    from concourse.bass_utils import run_bass_kernel_spmd

    nc = _get_prog()
    in_maps, ctx = host_prep(emp_samples, log_kde_rhos, x, y, eps, rand_idxs)

    trace = bool(int(os.environ.get("BNN_TRACE", "0")))
    try:
        res = run_bass_kernel_spmd(nc, in_maps, core_ids=list(range(N_CORES)),
                                   trace=trace)
    except ModuleNotFoundError:
        # NTFF profile hook unavailable in this container; run untraced.
        res = run_bass_kernel_spmd(nc, in_maps, core_ids=list(range(N_CORES)))
    LAST_EXEC_NS = res.exec_time_ns

    def _qsum(arr):
        arr = arr.astype(np.float64)
        main = arr[:, :TILES * NCHUNK].reshape(P, TILES, NCHUNK).sum(axis=2)
        main[:, 0] += arr[:, TILES * NCHUNK:].sum(axis=1)
        return main.T.reshape(N_LOC)

    qsum = np.concatenate([_qsum(r["qparts"]) for r in res.results])
    sv2 = np.concatenate(
        [r["sv2"].astype(np.float64).T.reshape(N_LOC) for r in res.results])
    samr = np.concatenate(
        [r["samr"].astype(np.float64).T.reshape(N_LOC) for r in res.results])
    return host_combine(ctx, qsum, sv2, samr)



# revision 10
# speedup vs baseline: 1.6544x; 1.6544x over previous
"""BNN-KDE ELBO kernel for Trainium2, data-parallel over 8192 samples on 8 cores.

Math (matches the jax reference to ~1e-4 rel):
  out = data_lp - kl_term
  kl_term  = mean_n [ m_n + log qsum_n - log K - prior_lp_n ]
  qsum_n   = sum_k exp(comp_lp[n,k] - m_n),  m_n = comp_lp[n, rand_idx_n]
  data_lp  = -0.5*B*mean_n ssq_n + B_X*0.5*(log B - log 2pi)
  ssq_n    = sum_b (y_pred[n](x_b) - y_b)^2

Device work per core (1024 samples = 8 tiles of 128 partitions):
  KDE: one PE matmul (contract 16, f32r) per 512-col block produces
    s[n,k] = A16*(comp_lp[n,k] - m_n) + C16 directly in PSUM (the affine
    Schraudolph transform rides extra lhsT rows). Row sums of exp then split
    across two engines to halve the serial exp cost:
      - ACT chunks: activation(Exp, scale=1/A16, bias=-C16/A16, accum_out)
      - DVE chunks: tensor_scalar(max,0)->int16 then a 4x-rate bf16-bitcast
        pass with accum_out: the int16 bits ARE bf16 exp values (Schraudolph);
        a host-side constant kappa corrects the known multiplicative bias.
  MLP: y_pred is a smooth 1-D function of x, so ssq_n is evaluated through a
    127-point Chebyshev grid: ssq_n = c_n^T G c_n + r.c_n + sum(y^2) with
    G = Phi^T Phi, r = -2 Phi^T y precomputed on host (Phi = barycentric
    interpolation matrix from nodes to the 2048 x points; exact to ~1e-4).
    Device: tiny bf16 DVE/ACT MLP at the nodes -> Cs[128,128], DMA-transpose,
    M = G*Cs^T + r (PE), usq = Cs^T . M (DVE), column sums via ones-matmul
    into one PSUM row per tile.
Host: O(N*D + B*Q^2) prep (gather, transposes, Chebyshev quadratic form) and
  the final scalar combine of per-core partial sums.
"""

import os
import sys

import numpy as np
import ml_dtypes
ml_bf16 = ml_dtypes.bfloat16

for _p in ("/opt/trn_rl_repo",):
    if _p not in sys.path and os.path.isdir(_p):
        sys.path.insert(0, _p)

NUM_NODES = 2
ALPHA = 1.0
BETA = 5.0
KL_BETA = 1.0
LOG_2PI = float(np.log(2.0 * np.pi))

K_COMP = 8192
N_SAMP = 8192
B_X = 2048
D_W = 13

N_CORES = 8
N_LOC = N_SAMP // N_CORES          # 1024 samples per core
P = 128                             # partitions
TILES = N_LOC // P                  # 8 sample-tiles per core
KSUB = 512                          # matmul free-dim granularity

Q = 127                             # Chebyshev nodes
QA = 128                            # padded quadratic-form size

# Schraudolph bf16 exp constants: int16 bits = max(A16*t + C16, 0) give a
# bf16 value ~ exp(t) with a stable multiplicative bias corrected by KAPPA.
A16 = 128.0 / float(np.log(2.0))
C16 = 16218.0
KAPPA = 1.1806

# pcol column indices
_C_W10, _C_W11, _C_B10, _C_B11 = 0, 1, 2, 3
_C_W200, _C_W201, _C_W210, _C_W211 = 4, 5, 6, 7
_C_B20, _C_B21, _C_W30, _C_W31, _C_B3 = 8, 9, 10, 11, 12
PCOLS = 13

# KDE chunking: 6 chunks per tile, 5x1536 + 1x512, split between ACT / DVE.
CHUNK_STARTS = [0, 1536, 3072, 4608, 6144, 7680]
CHUNK_SIZES = [1536, 1536, 1536, 1536, 1536, 512]


def act_chunks(t):
    return (0, 1, 2) if t % 2 == 0 else (0, 1, 2, 5)


def dve_chunks(t):
    return (3, 4, 5) if t % 2 == 0 else (3, 4)


def _col_maps():
    amap, dmap = {}, {}
    ac = dc = 0
    for t in range(TILES):
        for ch in act_chunks(t):
            amap[(t, ch)] = ac
            ac += 1
        for ch in dve_chunks(t):
            dmap[(t, ch)] = dc
            dc += 1
    return amap, dmap, ac, dc


ACT_COL, DVE_COL, N_ACT_COLS, N_DVE_COLS = _col_maps()

_PROG = None
LAST_EXEC_NS = None


def build_program():
    import concourse.bass as bass
    import concourse.tile as tile
    from concourse import bacc, mybir
    from concourse.bass_isa import ReduceOp

    f32 = mybir.dt.float32
    f32r = mybir.dt.float32r
    bf16 = mybir.dt.bfloat16
    i16 = mybir.dt.int16
    Alu = mybir.AluOpType
    Act = mybir.ActivationFunctionType

    nc = bacc.Bacc("TRN2", target_bir_lowering=False, debug=False,
                   num_devices=N_CORES)

    empT_d = nc.declare_dram_parameter("empT", [16, K_COMP], f32r, isOutput=False)
    wT_d = nc.declare_dram_parameter("wT", [16, N_LOC], f32r, isOutput=False)
    pcol_d = nc.declare_dram_parameter("pcol", [N_LOC, PCOLS], f32, isOutput=False)
    nodes_d = nc.declare_dram_parameter("nodes", [Q], bf16, isOutput=False)
    gmat_d = nc.declare_dram_parameter("gmat", [QA, QA], bf16, isOutput=False)
    rvec_d = nc.declare_dram_parameter("rvec", [1, QA], bf16, isOutput=False)
    qact_d = nc.declare_dram_parameter("qact", [P, N_ACT_COLS], f32, isOutput=True)
    qdve_d = nc.declare_dram_parameter("qdve", [P, N_DVE_COLS], f32, isOutput=True)
    ssq_d = nc.declare_dram_parameter("ssq", [TILES, P], f32, isOutput=True)

    exp_scale = float(1.0 / A16)
    exp_bias = float(-C16 / A16)

    with tile.TileContext(nc) as tc:
        with (
            tc.tile_pool(name="const", bufs=1) as cpool,
            tc.tile_pool(name="i16p", bufs=2) as ipool,
            tc.tile_pool(name="mlpa", bufs=2) as mpool,
            tc.tile_pool(name="mlpb", bufs=2) as m2pool,
            tc.tile_pool(name="psum", bufs=2, space=bass.MemorySpace.PSUM) as ppool,
            tc.tile_pool(name="psum1", bufs=1, space=bass.MemorySpace.PSUM) as p1pool,
        ):
            # ---- constants / inputs ----
            empT = cpool.tile([16, K_COMP], f32r)
            for s, sz in zip(CHUNK_STARTS, CHUNK_SIZES):
                nc.sync.dma_start(empT[:, s:s + sz], empT_d[:, s:s + sz])
            wT = cpool.tile([16, N_LOC], f32r)
            nc.sync.dma_start(wT[:], wT_d[:])

            warm = cpool.tile([P, 1], f32)
            nc.vector.memset(warm[:], 0.0)
            nc.scalar.activation(warm[:], warm[:], Act.Exp)
            ebias = cpool.tile([P, 1], f32)
            nc.vector.memset(ebias[:], exp_bias)

            pcs = []
            for t in range(TILES):
                pc = cpool.tile([P, PCOLS], f32, tag=f"pc{t}")
                nc.sync.dma_start(pc[:], pcol_d[t * P:(t + 1) * P, :])
                pcs.append(pc)

            nodes = cpool.tile([P, Q], bf16)
            nc.sync.dma_start(nodes[:], nodes_d[:].partition_broadcast(P))
            gmat = cpool.tile([QA, QA], bf16)
            nc.sync.dma_start(gmat[:], gmat_d[:])
            rvec = cpool.tile([1, QA], bf16)
            nc.sync.dma_start(rvec[:], rvec_d[:])
            ones_r = cpool.tile([1, QA], bf16)
            nc.vector.memset(ones_r[:], 1.0)
            ones_c = cpool.tile([P, 1], bf16)
            nc.vector.memset(ones_c[:], 1.0)

            qact_sb = cpool.tile([P, N_ACT_COLS], f32)
            qdve_sb = cpool.tile([P, N_DVE_COLS], f32)

            def emit_mlp(t):
                pc = pcs[t]
                # layer 1: per-node affine on nodes, then one tanh
                harg = mpool.tile([P, 2 * Q], bf16, tag="harg")
                for i in range(2):
                    nc.vector.tensor_scalar(
                        harg[:, i * Q:(i + 1) * Q], nodes[:],
                        pc[:, _C_W10 + i:_C_W10 + i + 1],
                        pc[:, _C_B10 + i:_C_B10 + i + 1],
                        Alu.mult, Alu.add)
                h = mpool.tile([P, 2 * Q], bf16, tag="h")
                nc.scalar.activation(h[:], harg[:], Act.Tanh)
                h0 = h[:, :Q]
                h1 = h[:, Q:]
                # layer 2
                garg = mpool.tile([P, 2 * Q], bf16, tag="garg")
                for i in range(2):
                    ti = m2pool.tile([P, Q], bf16, tag="ti")
                    nc.vector.tensor_scalar(
                        ti[:], h1,
                        pc[:, _C_W201 + 2 * i:_C_W201 + 2 * i + 1],
                        pc[:, _C_B20 + i:_C_B20 + i + 1],
                        Alu.mult, Alu.add)
                    nc.vector.scalar_tensor_tensor(
                        garg[:, i * Q:(i + 1) * Q], h0,
                        pc[:, _C_W200 + 2 * i:_C_W200 + 2 * i + 1],
                        ti[:], Alu.mult, Alu.add)
                g = mpool.tile([P, 2 * Q], bf16, tag="g")
                nc.scalar.activation(g[:], garg[:], Act.Tanh)
                # layer 3 -> Cs (incl b3), pad col 127 with zeros
                t3 = m2pool.tile([P, Q], bf16, tag="t3")
                nc.vector.tensor_scalar(
                    t3[:], g[:, :Q],
                    pc[:, _C_W30:_C_W30 + 1],
                    pc[:, _C_B3:_C_B3 + 1],
                    Alu.mult, Alu.add)
                cs = m2pool.tile([P, QA], bf16, tag="cs")
                nc.vector.scalar_tensor_tensor(
                    cs[:, :Q], g[:, Q:],
                    pc[:, _C_W31:_C_W31 + 1],
                    t3[:], Alu.mult, Alu.add)
                nc.vector.memset(cs[:, Q:QA], 0.0)
                # quadratic form: ssq_n = cs_n^T G cs_n + r . cs_n
                cts = m2pool.tile([QA, P], bf16, tag="cts")
                nc.sync.dma_start_transpose(cts[:], cs[:])
                mp = p1pool.tile([QA, P], f32, tag="mp")
                nc.tensor.matmul(mp[:], gmat[:], cts[:], start=True, stop=False)
                nc.tensor.matmul(mp[:], rvec[:], ones_r[:], start=False, stop=True)
                usq = m2pool.tile([QA, P], bf16, tag="usq")
                nc.vector.tensor_tensor(usq[:], cts[:], mp[:], Alu.mult)
                sred = m2pool.tile([QA, P], f32, tag="sred")
                nc.gpsimd.partition_all_reduce(sred[:], usq[:], P, ReduceOp.add)
                nc.sync.dma_start(ssq_d[t:t + 1, :], sred[0:1, :])

            def emit_kde(t):
                lhsT = wT[:, t * P:(t + 1) * P]
                achunks = act_chunks(t)
                for c, (k0, sz) in enumerate(zip(CHUNK_STARTS, CHUNK_SIZES)):
                    ps = ppool.tile([P, 1536], f32, tag="ps",
                                    space=bass.MemorySpace.PSUM)
                    for s in range(sz // KSUB):
                        nc.tensor.matmul(
                            ps[:, s * KSUB:(s + 1) * KSUB],
                            lhsT,
                            empT[:, k0 + s * KSUB:k0 + (s + 1) * KSUB],
                            start=True, stop=True)
                    if c in achunks:
                        col = ACT_COL[(t, c)]
                        nc.scalar.activation(
                            ps[:, :sz], ps[:, :sz], Act.Exp,
                            bias=ebias[:], scale=exp_scale,
                            accum_out=qact_sb[:, col:col + 1])
                    else:
                        col = DVE_COL[(t, c)]
                        it = ipool.tile([P, 1536], i16, tag="it")
                        nc.vector.tensor_scalar(
                            it[:, :sz], ps[:, :sz], 0.0, None, Alu.max)
                        bv = it[:, :sz].bitcast(bf16)
                        nc.vector.tensor_scalar(
                            bv, bv, 1.0, 0.0, Alu.mult, Alu.add,
                            accum_out=qdve_sb[:, col:col + 1])

            for t in range(TILES):
                emit_mlp(t)
                emit_kde(t)

            nc.sync.dma_start(qact_d[:], qact_sb[:])
            nc.sync.dma_start(qdve_d[:], qdve_sb[:])

    nc.compile()
    return nc


def _get_prog():
    global _PROG
    if _PROG is None:
        _PROG = build_program()
    return _PROG


def host_prep(emp_samples, log_kde_rhos, x, y, eps, rand_idxs):
    """Returns (per-core in_maps, host-side combine context)."""
    emp = np.asarray(emp_samples, np.float32)
    logr = np.asarray(log_kde_rhos, np.float32)
    x = np.asarray(x, np.float64).reshape(-1)
    y = np.asarray(y, np.float64).reshape(-1)
    eps = np.asarray(eps, np.float32)
    idx = np.asarray(rand_idxs).astype(np.int64)

    # softplus in f32, matching jax.nn.softplus
    kde_std = np.logaddexp(np.float32(0.0), logr).astype(np.float32)
    kde_var = (kde_std * kde_std).astype(np.float32)

    esq = np.einsum("kd,kd->k", emp, emp, dtype=np.float32).astype(np.float32)
    colconst = (-0.5 * (D_W * LOG_2PI + D_W * np.log(kde_var))).astype(np.float32)
    a = (-0.5 / kde_var).astype(np.float32)

    A = np.float32(A16)
    empT = np.empty((16, K_COMP), np.float32)
    empT[:D_W] = (A * emp / kde_var[:, None]).T
    empT[D_W] = A * a
    empT[D_W + 1] = A * (colconst + a * esq)
    empT[D_W + 2] = 1.0

    std_g = kde_std[idx]
    w = (emp[idx] + eps * std_g[:, None]).astype(np.float32)
    wsq = np.einsum("nd,nd->n", w, w, dtype=np.float32).astype(np.float32)
    epssq = np.einsum("nd,nd->n", eps, eps, dtype=np.float32)
    m = (colconst[idx] - 0.5 * epssq).astype(np.float32)

    # Chebyshev grid on the x range and the quadratic form for
    # ssq = |Phi c - y|^2 (Phi: barycentric interpolation matrix).
    lo, hi = x.min(), x.max()
    kk = np.arange(Q)
    tch = np.cos(np.pi * kk / (Q - 1))[::-1]
    nodes = (lo + hi) / 2 + (hi - lo) / 2 * tch
    bw = np.ones(Q)
    bw[0] = bw[-1] = 0.5
    bw *= (-1.0) ** kk
    diff = x[:, None] - nodes[None, :]
    hit = np.abs(diff) < 1e-13
    with np.errstate(divide="ignore", invalid="ignore"):
        tmp = bw[None, :] / diff
        Phi = tmp / tmp.sum(1)[:, None]
    rows_hit = hit.any(1)
    Phi[rows_hit] = hit[rows_hit].astype(np.float64)

    G = np.zeros((QA, QA), np.float64)
    G[:Q, :Q] = Phi.T @ Phi
    r2 = np.zeros((1, QA), np.float64)
    r2[0, :Q] = -2.0 * (Phi.T @ y)
    sy2 = float((y * y).sum())

    gmat = G.astype(ml_bf16)
    rvec = r2.astype(ml_bf16)
    nodes_b = nodes.astype(ml_bf16)

    in_maps = []
    for c in range(N_CORES):
        sl = slice(c * N_LOC, (c + 1) * N_LOC)
        wTc = np.empty((16, N_LOC), np.float32)
        wTc[:D_W] = w[sl].T
        wTc[D_W] = wsq[sl]
        wTc[D_W + 1] = 1.0
        wTc[D_W + 2] = np.float32(C16) - A * m[sl]
        in_maps.append({
            "empT": np.ascontiguousarray(empT),
            "wT": np.ascontiguousarray(wTc),
            "pcol": np.ascontiguousarray(w[sl]),
            "nodes": nodes_b,
            "gmat": gmat,
            "rvec": rvec,
        })

    ctx = {"wsq": wsq, "m": m, "sy2": sy2}
    return in_maps, ctx


def host_combine(ctx, qsum, ssq_dev):
    m = ctx["m"].astype(np.float64)
    wsq = ctx["wsq"].astype(np.float64)

    q_lp = m + np.log(np.maximum(qsum, 1e-300)) - np.log(float(K_COMP))
    prior_lp = -0.5 * ALPHA * wsq + D_W * 0.5 * (np.log(ALPHA) - LOG_2PI)
    kl_term = (q_lp - prior_lp).mean()

    ssq = ssq_dev + ctx["sy2"]
    data_lp = (-0.5 * BETA) * ssq.mean() + B_X * 0.5 * (np.log(BETA) - LOG_2PI)
    return np.float32(data_lp - KL_BETA * kl_term)


def kernel(emp_samples, log_kde_rhos, x, y, eps, rand_idxs):
    global LAST_EXEC_NS
    from concourse.bass_utils import run_bass_kernel_spmd

    nc = _get_prog()
    in_maps, ctx = host_prep(emp_samples, log_kde_rhos, x, y, eps, rand_idxs)

    trace = bool(int(os.environ.get("BNN_TRACE", "0")))
    try:
        res = run_bass_kernel_spmd(nc, in_maps, core_ids=list(range(N_CORES)),
                                   trace=trace)
    except ModuleNotFoundError:
        res = run_bass_kernel_spmd(nc, in_maps, core_ids=list(range(N_CORES)))
    LAST_EXEC_NS = res.exec_time_ns

    qsum_parts = []
    ssq_parts = []
    for r in res.results:
        qa = r["qact"].astype(np.float64)
        qd = r["qdve"].astype(np.float64)
        qsum_loc = np.empty(N_LOC, np.float64)
        for t in range(TILES):
            tot = np.zeros(P, np.float64)
            for ch in act_chunks(t):
                tot += qa[:, ACT_COL[(t, ch)]]
            dtot = np.zeros(P, np.float64)
            for ch in dve_chunks(t):
                dtot += qd[:, DVE_COL[(t, ch)]]
            qsum_loc[t * P:(t + 1) * P] = tot + KAPPA * dtot
        qsum_parts.append(qsum_loc)
        ssq_parts.append(r["ssq"].astype(np.float64).reshape(N_LOC))

    qsum = np.concatenate(qsum_parts)
    ssq_dev = np.concatenate(ssq_parts)
    return host_combine(ctx, qsum, ssq_dev)


# revision 14
# speedup vs baseline: 1.9147x; 1.1573x over previous
"""BNN-KDE ELBO kernel for Trainium2, data-parallel over 8192 samples on 8 cores.

Math (matches the jax reference to ~1e-4 rel):
  out = data_lp - kl_term
  kl_term  = mean_n [ m_n + log qsum_n - log K - prior_lp_n ]
  qsum_n   = sum_k exp(comp_lp[n,k] - m_n),  m_n = comp_lp[n, rand_idx_n]
  data_lp  = -0.5*B*mean_n ssq_n + B_X*0.5*(log B - log 2pi)
  ssq_n    = sum_b (y_pred[n](x_b) - y_b)^2

Device work per core (1024 samples = 8 tiles of 128 partitions):
  KDE: one PE matmul (contract 16, f32r) per 512-col block produces
    s[n,k] = A16*(comp_lp[n,k] - m_n) + C16 directly in PSUM (the affine
    Schraudolph transform rides extra lhsT rows). Row sums of exp then split
    across two engines to halve the serial exp cost:
      - ACT chunks: activation(Exp, scale=1/A16, bias=-C16/A16, accum_out)
      - DVE chunks: tensor_scalar(max,0)->int16 then a 4x-rate bf16-bitcast
        pass with accum_out: the int16 bits ARE bf16 exp values (Schraudolph);
        a host-side constant kappa corrects the known multiplicative bias.
  MLP: y_pred is a smooth 1-D function of x, so ssq_n is evaluated through a
    127-point Chebyshev grid: ssq_n = c_n^T G c_n + r.c_n + sum(y^2) with
    G = Phi^T Phi, r = -2 Phi^T y precomputed on host (Phi = barycentric
    interpolation matrix from nodes to the 2048 x points; exact to ~1e-4).
    Device: tiny bf16 DVE/ACT MLP at the nodes -> Cs[128,128], DMA-transpose,
    M = G*Cs^T + r (PE), usq = Cs^T . M (DVE), column sums via ones-matmul
    into one PSUM row per tile.
Host: O(N*D + B*Q^2) prep (gather, transposes, Chebyshev quadratic form) and
  the final scalar combine of per-core partial sums.
"""

import os
import sys

import numpy as np
import ml_dtypes
ml_bf16 = ml_dtypes.bfloat16

for _p in ("/opt/trn_rl_repo",):
    if _p not in sys.path and os.path.isdir(_p):
        sys.path.insert(0, _p)

NUM_NODES = 2
ALPHA = 1.0
BETA = 5.0
KL_BETA = 1.0
LOG_2PI = float(np.log(2.0 * np.pi))

K_COMP = 8192
N_SAMP = 8192
B_X = 2048
D_W = 13

N_CORES = 8
N_LOC = N_SAMP // N_CORES          # 1024 samples per core
P = 128                             # partitions
TILES = N_LOC // P                  # 8 sample-tiles per core
KSUB = 512                          # matmul free-dim granularity

Q = 127                             # Chebyshev nodes
QA = 128                            # padded quadratic-form size

# Schraudolph bf16 exp constants: int16 bits = max(A16*t + C16, 0) give a
# bf16 value ~ exp(t) with a stable multiplicative bias corrected by KAPPA.
A16 = 128.0 / float(np.log(2.0))
C16 = 16218.0
KAPPA = 1.1806

# pcol column indices
_C_W10, _C_W11, _C_B10, _C_B11 = 0, 1, 2, 3
_C_W200, _C_W201, _C_W210, _C_W211 = 4, 5, 6, 7
_C_B20, _C_B21, _C_W30, _C_W31, _C_B3 = 8, 9, 10, 11, 12
PCOLS = 13

# KDE chunking: 6 chunks per tile, 5x1536 + 1x512, split between ACT / DVE.
CHUNK_STARTS = [0, 1536, 3072, 4608, 6144, 7680]
CHUNK_SIZES = [1536, 1536, 1536, 1536, 1536, 512]


# Interleave ACT/DVE chunk ownership so both engines stream concurrently
# through the 2-buffer PSUM ring (even chunks land in buffer A, odd in B).
def act_chunks(t):
    return (0, 2, 4) if t % 2 == 0 else (0, 2, 4, 5)


def dve_chunks(t):
    return (1, 3, 5) if t % 2 == 0 else (1, 3)


def _col_maps():
    amap, dmap = {}, {}
    ac = dc = 0
    for t in range(TILES):
        for ch in act_chunks(t):
            amap[(t, ch)] = ac
            ac += 1
        for ch in dve_chunks(t):
            dmap[(t, ch)] = dc
            dc += 1
    return amap, dmap, ac, dc


ACT_COL, DVE_COL, N_ACT_COLS, N_DVE_COLS = _col_maps()

_PROG = None
LAST_EXEC_NS = None


def build_program():
    import concourse.bass as bass
    import concourse.tile as tile
    from concourse import bacc, mybir
    from concourse.bass_isa import ReduceOp

    f32 = mybir.dt.float32
    f32r = mybir.dt.float32r
    bf16 = mybir.dt.bfloat16
    i16 = mybir.dt.int16
    Alu = mybir.AluOpType
    Act = mybir.ActivationFunctionType

    nc = bacc.Bacc("TRN2", target_bir_lowering=False, debug=False,
                   num_devices=N_CORES)

    empT_d = nc.declare_dram_parameter("empT", [16, K_COMP], f32r, isOutput=False)
    wT_d = nc.declare_dram_parameter("wT", [16, N_LOC], f32r, isOutput=False)
    pcol_d = nc.declare_dram_parameter("pcol", [P, TILES * PCOLS], f32, isOutput=False)
    nodes_d = nc.declare_dram_parameter("nodes", [P, Q], bf16, isOutput=False)
    gmat_d = nc.declare_dram_parameter("gmat", [QA, QA], bf16, isOutput=False)
    rvec_d = nc.declare_dram_parameter("rvec", [1, QA], bf16, isOutput=False)
    qact_d = nc.declare_dram_parameter("qact", [P, N_ACT_COLS], f32, isOutput=True)
    qdve_d = nc.declare_dram_parameter("qdve", [P, N_DVE_COLS], f32, isOutput=True)
    ssq_d = nc.declare_dram_parameter("ssq", [TILES, P], f32, isOutput=True)

    exp_scale = float(1.0 / A16)
    exp_bias = float(-C16 / A16)

    with tile.TileContext(nc) as tc:
        with (
            tc.tile_pool(name="const", bufs=1) as cpool,
            tc.tile_pool(name="i16p", bufs=2) as ipool,
            tc.tile_pool(name="mlpa", bufs=2) as mpool,
            tc.tile_pool(name="mlpb", bufs=2) as m2pool,
            tc.tile_pool(name="psum", bufs=2, space=bass.MemorySpace.PSUM) as ppool,
            tc.tile_pool(name="psum1", bufs=1, space=bass.MemorySpace.PSUM) as p1pool,
        ):
            # ---- constants / inputs ----
            empT = cpool.tile([16, K_COMP], f32r)
            for s, e in ((0, 1536), (1536, 4608), (4608, K_COMP)):
                nc.sync.dma_start(empT[:, s:e], empT_d[:, s:e])
            wT = cpool.tile([16, N_LOC], f32r)
            nc.sync.dma_start(wT[:], wT_d[:])

            warm = cpool.tile([P, 1], f32)
            nc.vector.memset(warm[:], 0.0)
            nc.scalar.activation(warm[:], warm[:], Act.Exp)
            ebias = cpool.tile([P, 1], f32)
            nc.vector.memset(ebias[:], exp_bias)

            pcall = cpool.tile([P, TILES * PCOLS], f32)
            nc.sync.dma_start(pcall[:], pcol_d[:])
            pcs = [pcall[:, t * PCOLS:(t + 1) * PCOLS] for t in range(TILES)]

            nodes = cpool.tile([P, Q], bf16)
            nc.sync.dma_start(nodes[:], nodes_d[:])
            gmat = cpool.tile([QA, QA], bf16)
            nc.sync.dma_start(gmat[:], gmat_d[:])
            rvec = cpool.tile([1, QA], bf16)
            nc.sync.dma_start(rvec[:], rvec_d[:])
            ones_r = cpool.tile([1, QA], bf16)
            nc.vector.memset(ones_r[:], 1.0)
            ones_c = cpool.tile([P, 1], bf16)
            nc.vector.memset(ones_c[:], 1.0)

            qact_sb = cpool.tile([P, N_ACT_COLS], f32)
            qdve_sb = cpool.tile([P, N_DVE_COLS], f32)

            def emit_mlp(t):
                pc = pcs[t]
                # layer 1: per-node affine on nodes, then one tanh
                harg = mpool.tile([P, 2 * Q], bf16, tag="harg")
                for i in range(2):
                    nc.vector.tensor_scalar(
                        harg[:, i * Q:(i + 1) * Q], nodes[:],
                        pc[:, _C_W10 + i:_C_W10 + i + 1],
                        pc[:, _C_B10 + i:_C_B10 + i + 1],
                        Alu.mult, Alu.add)
                h = mpool.tile([P, 2 * Q], bf16, tag="h")
                nc.scalar.activation(h[:], harg[:], Act.Tanh)
                h0 = h[:, :Q]
                h1 = h[:, Q:]
                # layer 2
                garg = mpool.tile([P, 2 * Q], bf16, tag="garg")
                for i in range(2):
                    ti = m2pool.tile([P, Q], bf16, tag="ti")
                    nc.vector.tensor_scalar(
                        ti[:], h1,
                        pc[:, _C_W201 + 2 * i:_C_W201 + 2 * i + 1],
                        pc[:, _C_B20 + i:_C_B20 + i + 1],
                        Alu.mult, Alu.add)
                    nc.vector.scalar_tensor_tensor(
                        garg[:, i * Q:(i + 1) * Q], h0,
                        pc[:, _C_W200 + 2 * i:_C_W200 + 2 * i + 1],
                        ti[:], Alu.mult, Alu.add)
                g = mpool.tile([P, 2 * Q], bf16, tag="g")
                nc.scalar.activation(g[:], garg[:], Act.Tanh)
                # layer 3 -> Cs (incl b3), pad col 127 with zeros
                t3 = m2pool.tile([P, Q], bf16, tag="t3")
                nc.vector.tensor_scalar(
                    t3[:], g[:, :Q],
                    pc[:, _C_W30:_C_W30 + 1],
                    pc[:, _C_B3:_C_B3 + 1],
                    Alu.mult, Alu.add)
                cs = m2pool.tile([P, QA], bf16, tag="cs")
                nc.vector.scalar_tensor_tensor(
                    cs[:, :Q], g[:, Q:],
                    pc[:, _C_W31:_C_W31 + 1],
                    t3[:], Alu.mult, Alu.add)
                nc.vector.memset(cs[:, Q:QA], 0.0)
                # quadratic form: ssq_n = cs_n^T G cs_n + r . cs_n
                cts = m2pool.tile([QA, P], bf16, tag="cts")
                nc.sync.dma_start_transpose(cts[:], cs[:])
                mp = p1pool.tile([QA, P], f32, tag="mp")
                nc.tensor.matmul(mp[:], gmat[:], cts[:], start=True, stop=False)
                nc.tensor.matmul(mp[:], rvec[:], ones_r[:], start=False, stop=True)
                usq = m2pool.tile([QA, P], bf16, tag="usq")
                nc.vector.tensor_tensor(usq[:], cts[:], mp[:], Alu.mult)
                sred = m2pool.tile([QA, P], f32, tag="sred")
                nc.gpsimd.partition_all_reduce(sred[:], usq[:], P, ReduceOp.add)
                nc.sync.dma_start(ssq_d[t:t + 1, :], sred[0:1, :])

            def emit_kde(t):
                lhsT = wT[:, t * P:(t + 1) * P]
                achunks = act_chunks(t)
                for c, (k0, sz) in enumerate(zip(CHUNK_STARTS, CHUNK_SIZES)):
                    ps = ppool.tile([P, 1536], f32, tag="ps",
                                    space=bass.MemorySpace.PSUM)
                    for s in range(sz // KSUB):
                        nc.tensor.matmul(
                            ps[:, s * KSUB:(s + 1) * KSUB],
                            lhsT,
                            empT[:, k0 + s * KSUB:k0 + (s + 1) * KSUB],
                            start=True, stop=True)
                    if c in achunks:
                        col = ACT_COL[(t, c)]
                        nc.scalar.activation(
                            ps[:, :sz], ps[:, :sz], Act.Exp,
                            bias=ebias[:], scale=exp_scale,
                            accum_out=qact_sb[:, col:col + 1])
                    else:
                        col = DVE_COL[(t, c)]
                        it = ipool.tile([P, 1536], i16, tag="it")
                        nc.vector.tensor_scalar(
                            it[:, :sz], ps[:, :sz], 0.0, None, Alu.max)
                        bv = it[:, :sz].bitcast(bf16)
                        nc.vector.tensor_scalar(
                            bv, bv, 1.0, 0.0, Alu.mult, Alu.add,
                            accum_out=qdve_sb[:, col:col + 1])

            for t in range(TILES):
                emit_mlp(t)
                emit_kde(t)

            nc.sync.dma_start(qact_d[:], qact_sb[:])
            nc.sync.dma_start(qdve_d[:], qdve_sb[:])

    nc.compile()
    return nc


def _get_prog():
    global _PROG
    if _PROG is None:
        _PROG = build_program()
    return _PROG


def host_prep(emp_samples, log_kde_rhos, x, y, eps, rand_idxs):
    """Returns (per-core in_maps, host-side combine context)."""
    emp = np.asarray(emp_samples, np.float32)
    logr = np.asarray(log_kde_rhos, np.float32)
    x = np.asarray(x, np.float64).reshape(-1)
    y = np.asarray(y, np.float64).reshape(-1)
    eps = np.asarray(eps, np.float32)
    idx = np.asarray(rand_idxs).astype(np.int64)

    # softplus in f32, matching jax.nn.softplus
    kde_std = np.logaddexp(np.float32(0.0), logr).astype(np.float32)
    kde_var = (kde_std * kde_std).astype(np.float32)

    esq = np.einsum("kd,kd->k", emp, emp, dtype=np.float32).astype(np.float32)
    colconst = (-0.5 * (D_W * LOG_2PI + D_W * np.log(kde_var))).astype(np.float32)
    a = (-0.5 / kde_var).astype(np.float32)

    A = np.float32(A16)
    empT = np.empty((16, K_COMP), np.float32)
    empT[:D_W] = (A * emp / kde_var[:, None]).T
    empT[D_W] = A * a
    empT[D_W + 1] = A * (colconst + a * esq)
    empT[D_W + 2] = 1.0

    std_g = kde_std[idx]
    w = (emp[idx] + eps * std_g[:, None]).astype(np.float32)
    wsq = np.einsum("nd,nd->n", w, w, dtype=np.float32).astype(np.float32)
    epssq = np.einsum("nd,nd->n", eps, eps, dtype=np.float32)
    m = (colconst[idx] - 0.5 * epssq).astype(np.float32)

    # Chebyshev grid on the x range and the quadratic form for
    # ssq = |Phi c - y|^2 (Phi: barycentric interpolation matrix).
    lo, hi = x.min(), x.max()
    kk = np.arange(Q)
    tch = np.cos(np.pi * kk / (Q - 1))[::-1]
    nodes = (lo + hi) / 2 + (hi - lo) / 2 * tch
    bw = np.ones(Q)
    bw[0] = bw[-1] = 0.5
    bw *= (-1.0) ** kk
    diff = x[:, None] - nodes[None, :]
    hit = np.abs(diff) < 1e-13
    with np.errstate(divide="ignore", invalid="ignore"):
        tmp = bw[None, :] / diff
        Phi = tmp / tmp.sum(1)[:, None]
    rows_hit = hit.any(1)
    Phi[rows_hit] = hit[rows_hit].astype(np.float64)

    G = np.zeros((QA, QA), np.float64)
    G[:Q, :Q] = Phi.T @ Phi
    r2 = np.zeros((1, QA), np.float64)
    r2[0, :Q] = -2.0 * (Phi.T @ y)
    sy2 = float((y * y).sum())

    gmat = G.astype(ml_bf16)
    rvec = r2.astype(ml_bf16)
    nodes_b = np.ascontiguousarray(
        np.broadcast_to(nodes.astype(ml_bf16), (P, Q)))

    in_maps = []
    for c in range(N_CORES):
        sl = slice(c * N_LOC, (c + 1) * N_LOC)
        wTc = np.empty((16, N_LOC), np.float32)
        wTc[:D_W] = w[sl].T
        wTc[D_W] = wsq[sl]
        wTc[D_W + 1] = 1.0
        wTc[D_W + 2] = np.float32(C16) - A * m[sl]
        # pcol packed partition-major: [128, TILES*13]
        pcp = np.ascontiguousarray(
            w[sl].reshape(TILES, P, PCOLS).transpose(1, 0, 2).reshape(P, TILES * PCOLS))
        in_maps.append({
            "empT": np.ascontiguousarray(empT),
            "wT": np.ascontiguousarray(wTc),
            "pcol": pcp,
            "nodes": nodes_b,
            "gmat": gmat,
            "rvec": rvec,
        })

    ctx = {"wsq": wsq, "m": m, "sy2": sy2}
    return in_maps, ctx


def host_combine(ctx, qsum, ssq_dev):
    m = ctx["m"].astype(np.float64)
    wsq = ctx["wsq"].astype(np.float64)

    q_lp = m + np.log(np.maximum(qsum, 1e-300)) - np.log(float(K_COMP))
    prior_lp = -0.5 * ALPHA * wsq + D_W * 0.5 * (np.log(ALPHA) - LOG_2PI)
    kl_term = (q_lp - prior_lp).mean()

    ssq = ssq_dev + ctx["sy2"]
    data_lp = (-0.5 * BETA) * ssq.mean() + B_X * 0.5 * (np.log(BETA) - LOG_2PI)
    return np.float32(data_lp - KL_BETA * kl_term)


def kernel(emp_samples, log_kde_rhos, x, y, eps, rand_idxs):
    global LAST_EXEC_NS
    from concourse.bass_utils import run_bass_kernel_spmd

    nc = _get_prog()
    in_maps, ctx = host_prep(emp_samples, log_kde_rhos, x, y, eps, rand_idxs)

    trace = bool(int(os.environ.get("BNN_TRACE", "0")))
    try:
        res = run_bass_kernel_spmd(nc, in_maps, core_ids=list(range(N_CORES)),
                                   trace=trace)
    except ModuleNotFoundError:
        res = run_bass_kernel_spmd(nc, in_maps, core_ids=list(range(N_CORES)))
    LAST_EXEC_NS = res.exec_time_ns

    qsum_parts = []
    ssq_parts = []
    for r in res.results:
        qa = r["qact"].astype(np.float64)
        qd = r["qdve"].astype(np.float64)
        qsum_loc = np.empty(N_LOC, np.float64)
        for t in range(TILES):
            tot = np.zeros(P, np.float64)
            for ch in act_chunks(t):
                tot += qa[:, ACT_COL[(t, ch)]]
            dtot = np.zeros(P, np.float64)
            for ch in dve_chunks(t):
                dtot += qd[:, DVE_COL[(t, ch)]]
            qsum_loc[t * P:(t + 1) * P] = tot + KAPPA * dtot
        qsum_parts.append(qsum_loc)
        ssq_parts.append(r["ssq"].astype(np.float64).reshape(N_LOC))

    qsum = np.concatenate(qsum_parts)
    ssq_dev = np.concatenate(ssq_parts)
    return host_combine(ctx, qsum, ssq_dev)


# revision 39
# speedup vs baseline: 2.0077x; 1.0485x over previous
"""BNN-KDE ELBO kernel for Trainium2, data-parallel over 8192 samples on 8 cores.

Math (matches the jax reference to ~1e-4 rel):
  out = data_lp - kl_term
  kl_term  = mean_n [ m_n + log qsum_n - log K - prior_lp_n ]
  qsum_n   = sum_k exp(comp_lp[n,k] - m_n),  m_n = comp_lp[n, rand_idx_n]
  data_lp  = -0.5*B*mean_n ssq_n + B_X*0.5*(log B - log 2pi)
  ssq_n    = sum_b (y_pred[n](x_b) - y_b)^2

Device work per core (1024 samples = 8 tiles of 128 partitions):
  KDE: one PE matmul (contract 16, f32r) per 512-col block produces
    s[n,k] = A16*(comp_lp[n,k] - m_n) + C16 directly in PSUM (the affine
    Schraudolph transform rides extra lhsT rows). Row sums of exp then split
    across two engines to halve the serial exp cost:
      - ACT chunks: activation(Exp, scale=1/A16, bias=-C16/A16, accum_out)
      - DVE chunks: tensor_scalar(max,0)->int16 then a 4x-rate bf16-bitcast
        pass with accum_out: the int16 bits ARE bf16 exp values (Schraudolph);
        a host-side constant kappa corrects the known multiplicative bias.
  MLP: y_pred is a smooth 1-D function of x, so ssq_n is evaluated through a
    127-point Chebyshev grid: ssq_n = c_n^T G c_n + r.c_n + sum(y^2) with
    G = Phi^T Phi, r = -2 Phi^T y precomputed on host (Phi = barycentric
    interpolation matrix from nodes to the 2048 x points; exact to ~1e-4).
    Device: tiny bf16 DVE/ACT MLP at the nodes -> Cs[128,128], DMA-transpose,
    M = G*Cs^T + r (PE), usq = Cs^T . M (DVE), column sums via ones-matmul
    into one PSUM row per tile.
Host: O(N*D + B*Q^2) prep (gather, transposes, Chebyshev quadratic form) and
  the final scalar combine of per-core partial sums.
"""

import os
import sys

import numpy as np
import ml_dtypes
ml_bf16 = ml_dtypes.bfloat16

for _p in ("/opt/trn_rl_repo",):
    if _p not in sys.path and os.path.isdir(_p):
        sys.path.insert(0, _p)

NUM_NODES = 2
ALPHA = 1.0
BETA = 5.0
KL_BETA = 1.0
LOG_2PI = float(np.log(2.0 * np.pi))

K_COMP = 8192
N_SAMP = 8192
B_X = 2048
D_W = 13

N_CORES = 8
N_LOC = N_SAMP // N_CORES          # 1024 samples per core
P = 128                             # partitions
TILES = N_LOC // P                  # 8 sample-tiles per core
KSUB = 512                          # matmul free-dim granularity

Q = 127                             # Chebyshev nodes
QA = 128                            # padded quadratic-form size

# Schraudolph bf16 exp constants: int16 bits = max(A16*t + C16, 0) give a
# bf16 value ~ exp(t) with a stable multiplicative bias corrected by KAPPA.
A16 = 128.0 / float(np.log(2.0))
C16 = 16218.0
KAPPA = 1.1806

# pcol column indices
_C_W10, _C_W11, _C_B10, _C_B11 = 0, 1, 2, 3
_C_W200, _C_W201, _C_W210, _C_W211 = 4, 5, 6, 7
_C_B20, _C_B21, _C_W30, _C_W31, _C_B3 = 8, 9, 10, 11, 12
PCOLS = 13

# KDE chunking: 8 chunks of 1024 per tile through a 3-deep PSUM ring so the
# PE refill latency stays hidden; ownership interleaved A D A A D A A D to
# keep both engines streaming at the ~5:3 throughput ratio.
KCHUNK = 1024
CHUNK_STARTS = list(range(0, K_COMP, KCHUNK))
CHUNK_SIZES = [KCHUNK] * len(CHUNK_STARTS)


def act_chunks(t):
    return (0, 1, 2, 3, 4, 5, 6) if t == TILES - 1 else (0, 2, 3, 5, 6)


def dve_chunks(t):
    return (7,) if t == TILES - 1 else (1, 4, 7)


def _col_maps():
    amap, dmap = {}, {}
    ac = dc = 0
    for t in range(TILES):
        for ch in act_chunks(t):
            amap[(t, ch)] = ac
            ac += 1
        for ch in dve_chunks(t):
            dmap[(t, ch)] = dc
            dc += 1
    return amap, dmap, ac, dc


ACT_COL, DVE_COL, N_ACT_COLS, N_DVE_COLS = _col_maps()

_PROG = None
LAST_EXEC_NS = None


def build_program():
    import concourse.bass as bass
    import concourse.tile as tile
    from concourse import bacc, mybir
    from concourse.bass_isa import ReduceOp

    f32 = mybir.dt.float32
    f32r = mybir.dt.float32r
    bf16 = mybir.dt.bfloat16
    i16 = mybir.dt.int16
    Alu = mybir.AluOpType
    Act = mybir.ActivationFunctionType

    nc = bacc.Bacc("TRN2", target_bir_lowering=False, debug=False,
                   num_devices=N_CORES)

    empT_d = nc.declare_dram_parameter("empT", [16, K_COMP], f32r, isOutput=False)
    wT_d = nc.declare_dram_parameter("wT", [16, N_LOC], f32r, isOutput=False)
    pcol_d = nc.declare_dram_parameter("pcol", [P, TILES * PCOLS], f32, isOutput=False)
    nodes_d = nc.declare_dram_parameter("nodes", [P, Q], bf16, isOutput=False)
    gmat_d = nc.declare_dram_parameter("gmat", [QA, QA], bf16, isOutput=False)
    rvec_d = nc.declare_dram_parameter("rvec", [1, QA], bf16, isOutput=False)
    qact_d = nc.declare_dram_parameter("qact", [P, N_ACT_COLS], f32, isOutput=True)
    qdve_d = nc.declare_dram_parameter("qdve", [P, N_DVE_COLS], f32, isOutput=True)
    ssq_d = nc.declare_dram_parameter("ssq", [TILES, P], f32, isOutput=True)

    exp_scale = float(1.0 / A16)
    exp_bias = float(-C16 / A16)

    with tile.TileContext(nc) as tc:
        with (
            tc.tile_pool(name="const", bufs=1) as cpool,
            tc.tile_pool(name="i16p", bufs=3) as ipool,
            tc.tile_pool(name="mlpa", bufs=3) as mpool,
            tc.tile_pool(name="mlpb", bufs=3) as m2pool,
            tc.tile_pool(name="psum", bufs=3, space=bass.MemorySpace.PSUM) as ppool,
            tc.tile_pool(name="psum1", bufs=2, space=bass.MemorySpace.PSUM) as p1pool,
        ):
            # ---- constants / inputs (first pieces unblock tile-0 work) ----
            empT = cpool.tile([16, K_COMP], f32r)
            wT = cpool.tile([16, N_LOC], f32r)
            pcall = cpool.tile([P, TILES * PCOLS], f32)
            nodes = cpool.tile([P, Q], bf16)
            nc.sync.dma_start(empT[:, 0:2048], empT_d[:, 0:2048])
            nc.sync.dma_start(wT[:], wT_d[:])
            nc.sync.dma_start(pcall[:], pcol_d[:])
            nc.sync.dma_start(nodes[:], nodes_d[:])
            nc.sync.dma_start(empT[:, 2048:5120], empT_d[:, 2048:5120])
            nc.sync.dma_start(empT[:, 5120:K_COMP], empT_d[:, 5120:K_COMP])
            pcs = [pcall[:, t * PCOLS:(t + 1) * PCOLS] for t in range(TILES)]

            warm = cpool.tile([P, 1], f32)
            nc.vector.memset(warm[:], 0.0)
            nc.scalar.activation(warm[:], warm[:], Act.Exp)
            ebias = cpool.tile([P, 1], f32)
            nc.vector.memset(ebias[:], exp_bias)
            gmat = cpool.tile([QA, QA], bf16)
            nc.sync.dma_start(gmat[:], gmat_d[:])
            rvec = cpool.tile([1, QA], bf16)
            nc.sync.dma_start(rvec[:], rvec_d[:])
            ones_r = cpool.tile([1, QA], bf16)
            nc.vector.memset(ones_r[:], 1.0)
            ones_c = cpool.tile([P, 1], bf16)
            nc.vector.memset(ones_c[:], 1.0)

            qact_sb = cpool.tile([P, N_ACT_COLS], f32)
            qdve_sb = cpool.tile([P, N_DVE_COLS], f32)

            def emit_mlp(t):
                pc = pcs[t]
                # layer 1: per-node affine on nodes, then one tanh
                harg = mpool.tile([P, 2 * Q], bf16, tag="harg")
                for i in range(2):
                    nc.vector.tensor_scalar(
                        harg[:, i * Q:(i + 1) * Q], nodes[:],
                        pc[:, _C_W10 + i:_C_W10 + i + 1],
                        pc[:, _C_B10 + i:_C_B10 + i + 1],
                        Alu.mult, Alu.add)
                h = mpool.tile([P, 2 * Q], bf16, tag="h")
                nc.scalar.activation(h[:], harg[:], Act.Tanh)
                h0 = h[:, :Q]
                h1 = h[:, Q:]
                # layer 2
                garg = mpool.tile([P, 2 * Q], bf16, tag="garg")
                for i in range(2):
                    ti = m2pool.tile([P, Q], bf16, tag="ti")
                    nc.vector.tensor_scalar(
                        ti[:], h1,
                        pc[:, _C_W201 + 2 * i:_C_W201 + 2 * i + 1],
                        pc[:, _C_B20 + i:_C_B20 + i + 1],
                        Alu.mult, Alu.add)
                    nc.vector.scalar_tensor_tensor(
                        garg[:, i * Q:(i + 1) * Q], h0,
                        pc[:, _C_W200 + 2 * i:_C_W200 + 2 * i + 1],
                        ti[:], Alu.mult, Alu.add)
                g = mpool.tile([P, 2 * Q], bf16, tag="g")
                nc.scalar.activation(g[:], garg[:], Act.Tanh)
                # layer 3 -> Cs (incl b3), pad col 127 with zeros
                t3 = m2pool.tile([P, Q], bf16, tag="t3")
                nc.vector.tensor_scalar(
                    t3[:], g[:, :Q],
                    pc[:, _C_W30:_C_W30 + 1],
                    pc[:, _C_B3:_C_B3 + 1],
                    Alu.mult, Alu.add)
                cs = m2pool.tile([P, QA], bf16, tag="cs")
                nc.vector.scalar_tensor_tensor(
                    cs[:, :Q], g[:, Q:],
                    pc[:, _C_W31:_C_W31 + 1],
                    t3[:], Alu.mult, Alu.add)
                nc.vector.memset(cs[:, Q:QA], 0.0)
                # quadratic form: ssq_n = cs_n^T G cs_n + r . cs_n
                cts = m2pool.tile([QA, P], bf16, tag="cts")
                nc.sync.dma_start_transpose(cts[:], cs[:])
                mp = p1pool.tile([QA, P], f32, tag="mp")
                nc.tensor.matmul(mp[:], gmat[:], cts[:], start=True, stop=False)
                nc.tensor.matmul(mp[:], rvec[:], ones_r[:], start=False, stop=True)
                usq = m2pool.tile([QA, P], bf16, tag="usq")
                nc.vector.tensor_tensor(usq[:], cts[:], mp[:], Alu.mult)
                sred = m2pool.tile([QA, P], f32, tag="sred")
                nc.gpsimd.partition_all_reduce(sred[:], usq[:], P, ReduceOp.add)
                nc.sync.dma_start(ssq_d[t:t + 1, :], sred[0:1, :])

            def emit_kde(t):
                lhsT = wT[:, t * P:(t + 1) * P]
                achunks = act_chunks(t)
                for c, (k0, sz) in enumerate(zip(CHUNK_STARTS, CHUNK_SIZES)):
                    ps = ppool.tile([P, KCHUNK], f32, tag="ps",
                                    space=bass.MemorySpace.PSUM)
                    for s in range(sz // KSUB):
                        nc.tensor.matmul(
                            ps[:, s * KSUB:(s + 1) * KSUB],
                            lhsT,
                            empT[:, k0 + s * KSUB:k0 + (s + 1) * KSUB],
                            start=True, stop=True)
                    if c in achunks:
                        col = ACT_COL[(t, c)]
                        nc.scalar.activation(
                            ps[:, :sz], ps[:, :sz], Act.Exp,
                            bias=ebias[:], scale=exp_scale,
                            accum_out=qact_sb[:, col:col + 1])
                    else:
                        col = DVE_COL[(t, c)]
                        it = ipool.tile([P, KCHUNK], i16, tag="it")
                        nc.vector.tensor_scalar(
                            it[:, :sz], ps[:, :sz], 0.0, None, Alu.max)
                        bv = it[:, :sz].bitcast(bf16)
                        nc.vector.tensor_scalar(
                            bv, bv, 1.0, 0.0, Alu.mult, Alu.add,
                            accum_out=qdve_sb[:, col:col + 1])

            for t in range(TILES):
                emit_mlp(t)
                emit_kde(t)

            nc.sync.dma_start(qact_d[:], qact_sb[:])
            nc.sync.dma_start(qdve_d[:], qdve_sb[:])

    nc.compile()
    return nc


def _get_prog():
    global _PROG
    if _PROG is None:
        _PROG = build_program()
    return _PROG


def host_prep(emp_samples, log_kde_rhos, x, y, eps, rand_idxs):
    """Returns (per-core in_maps, host-side combine context)."""
    emp = np.asarray(emp_samples, np.float32)
    logr = np.asarray(log_kde_rhos, np.float32)
    x = np.asarray(x, np.float64).reshape(-1)
    y = np.asarray(y, np.float64).reshape(-1)
    eps = np.asarray(eps, np.float32)
    idx = np.asarray(rand_idxs).astype(np.int64)

    # softplus in f32, matching jax.nn.softplus
    kde_std = np.logaddexp(np.float32(0.0), logr).astype(np.float32)
    kde_var = (kde_std * kde_std).astype(np.float32)

    esq = np.einsum("kd,kd->k", emp, emp, dtype=np.float32).astype(np.float32)
    colconst = (-0.5 * (D_W * LOG_2PI + D_W * np.log(kde_var))).astype(np.float32)
    a = (-0.5 / kde_var).astype(np.float32)

    A = np.float32(A16)
    empT = np.empty((16, K_COMP), np.float32)
    empT[:D_W] = (A * emp / kde_var[:, None]).T
    empT[D_W] = A * a
    empT[D_W + 1] = A * (colconst + a * esq)
    empT[D_W + 2] = 1.0

    std_g = kde_std[idx]
    w = (emp[idx] + eps * std_g[:, None]).astype(np.float32)
    wsq = np.einsum("nd,nd->n", w, w, dtype=np.float32).astype(np.float32)
    epssq = np.einsum("nd,nd->n", eps, eps, dtype=np.float32)
    m = (colconst[idx] - 0.5 * epssq).astype(np.float32)

    # Chebyshev grid on the x range and the quadratic form for
    # ssq = |Phi c - y|^2 (Phi: barycentric interpolation matrix).
    lo, hi = x.min(), x.max()
    kk = np.arange(Q)
    tch = np.cos(np.pi * kk / (Q - 1))[::-1]
    nodes = (lo + hi) / 2 + (hi - lo) / 2 * tch
    bw = np.ones(Q)
    bw[0] = bw[-1] = 0.5
    bw *= (-1.0) ** kk
    diff = x[:, None] - nodes[None, :]
    hit = np.abs(diff) < 1e-13
    with np.errstate(divide="ignore", invalid="ignore"):
        tmp = bw[None, :] / diff
        Phi = tmp / tmp.sum(1)[:, None]
    rows_hit = hit.any(1)
    Phi[rows_hit] = hit[rows_hit].astype(np.float64)

    G = np.zeros((QA, QA), np.float64)
    G[:Q, :Q] = Phi.T @ Phi
    r2 = np.zeros((1, QA), np.float64)
    r2[0, :Q] = -2.0 * (Phi.T @ y)
    sy2 = float((y * y).sum())

    gmat = G.astype(ml_bf16)
    rvec = r2.astype(ml_bf16)
    nodes_b = np.ascontiguousarray(
        np.broadcast_to(nodes.astype(ml_bf16), (P, Q)))

    in_maps = []
    for c in range(N_CORES):
        sl = slice(c * N_LOC, (c + 1) * N_LOC)
        wTc = np.empty((16, N_LOC), np.float32)
        wTc[:D_W] = w[sl].T
        wTc[D_W] = wsq[sl]
        wTc[D_W + 1] = 1.0
        wTc[D_W + 2] = np.float32(C16) - A * m[sl]
        # pcol packed partition-major: [128, TILES*13]
        pcp = np.ascontiguousarray(
            w[sl].reshape(TILES, P, PCOLS).transpose(1, 0, 2).reshape(P, TILES * PCOLS))
        in_maps.append({
            "empT": np.ascontiguousarray(empT),
            "wT": np.ascontiguousarray(wTc),
            "pcol": pcp,
            "nodes": nodes_b,
            "gmat": gmat,
            "rvec": rvec,
        })

    ctx = {"wsq": wsq, "m": m, "sy2": sy2}
    return in_maps, ctx


def host_combine(ctx, qsum, ssq_dev):
    m = ctx["m"].astype(np.float64)
    wsq = ctx["wsq"].astype(np.float64)

    q_lp = m + np.log(np.maximum(qsum, 1e-300)) - np.log(float(K_COMP))
    prior_lp = -0.5 * ALPHA * wsq + D_W * 0.5 * (np.log(ALPHA) - LOG_2PI)
    kl_term = (q_lp - prior_lp).mean()

    ssq = ssq_dev + ctx["sy2"]
    data_lp = (-0.5 * BETA) * ssq.mean() + B_X * 0.5 * (np.log(BETA) - LOG_2PI)
    return np.float32(data_lp - KL_BETA * kl_term)


def kernel(emp_samples, log_kde_rhos, x, y, eps, rand_idxs):
    global LAST_EXEC_NS
    from concourse.bass_utils import run_bass_kernel_spmd

    nc = _get_prog()
    in_maps, ctx = host_prep(emp_samples, log_kde_rhos, x, y, eps, rand_idxs)

    trace = bool(int(os.environ.get("BNN_TRACE", "0")))
    try:
        res = run_bass_kernel_spmd(nc, in_maps, core_ids=list(range(N_CORES)),
                                   trace=trace)
    except ModuleNotFoundError:
        res = run_bass_kernel_spmd(nc, in_maps, core_ids=list(range(N_CORES)))
    LAST_EXEC_NS = res.exec_time_ns

    qsum_parts = []
    ssq_parts = []
    for r in res.results:
        qa = r["qact"].astype(np.float64)
        qd = r["qdve"].astype(np.float64)
        qsum_loc = np.empty(N_LOC, np.float64)
        for t in range(TILES):
            tot = np.zeros(P, np.float64)
            for ch in act_chunks(t):
                tot += qa[:, ACT_COL[(t, ch)]]
            dtot = np.zeros(P, np.float64)
            for ch in dve_chunks(t):
                dtot += qd[:, DVE_COL[(t, ch)]]
            qsum_loc[t * P:(t + 1) * P] = tot + KAPPA * dtot
        qsum_parts.append(qsum_loc)
        ssq_parts.append(r["ssq"].astype(np.float64).reshape(N_LOC))

    qsum = np.concatenate(qsum_parts)
    ssq_dev = np.concatenate(ssq_parts)
    return host_combine(ctx, qsum, ssq_dev)


# revision 53
# speedup vs baseline: 2.0244x; 1.0083x over previous
"""BNN-KDE ELBO kernel for Trainium2, data-parallel over 8192 samples on 8 cores.

Math (matches the jax reference to ~1e-4 rel):
  out = data_lp - kl_term
  kl_term  = mean_n [ m_n + log qsum_n - log K - prior_lp_n ]
  qsum_n   = sum_k exp(comp_lp[n,k] - m_n),  m_n = comp_lp[n, rand_idx_n]
  data_lp  = -0.5*B*mean_n ssq_n + B_X*0.5*(log B - log 2pi)
  ssq_n    = sum_b (y_pred[n](x_b) - y_b)^2

Device work per core (1024 samples = 8 tiles of 128 partitions):
  KDE: one PE matmul (contract 16, f32r) per 512-col block produces
    s[n,k] = A16*(comp_lp[n,k] - m_n) + C16 directly in PSUM (the affine
    Schraudolph transform rides extra lhsT rows). Row sums of exp then split
    across two engines to halve the serial exp cost:
      - ACT chunks: activation(Exp, scale=1/A16, bias=-C16/A16, accum_out)
      - DVE chunks: tensor_scalar(max,0)->int16 then a 4x-rate bf16-bitcast
        pass with accum_out: the int16 bits ARE bf16 exp values (Schraudolph);
        a host-side constant kappa corrects the known multiplicative bias.
  MLP: y_pred is a smooth 1-D function of x, so ssq_n is evaluated through a
    127-point Chebyshev grid: ssq_n = c_n^T G c_n + r.c_n + sum(y^2) with
    G = Phi^T Phi, r = -2 Phi^T y precomputed on host (Phi = barycentric
    interpolation matrix from nodes to the 2048 x points; exact to ~1e-4).
    Device: tiny bf16 DVE/ACT MLP at the nodes -> Cs[128,128], DMA-transpose,
    M = G*Cs^T + r (PE), usq = Cs^T . M (DVE), column sums via ones-matmul
    into one PSUM row per tile.
Host: O(N*D + B*Q^2) prep (gather, transposes, Chebyshev quadratic form) and
  the final scalar combine of per-core partial sums.
"""

import os
import sys

import numpy as np
import ml_dtypes
ml_bf16 = ml_dtypes.bfloat16

for _p in ("/opt/trn_rl_repo",):
    if _p not in sys.path and os.path.isdir(_p):
        sys.path.insert(0, _p)

NUM_NODES = 2
ALPHA = 1.0
BETA = 5.0
KL_BETA = 1.0
LOG_2PI = float(np.log(2.0 * np.pi))

K_COMP = 8192
N_SAMP = 8192
B_X = 2048
D_W = 13

N_CORES = 8
N_LOC = N_SAMP // N_CORES          # 1024 samples per core
P = 128                             # partitions
TILES = N_LOC // P                  # 8 sample-tiles per core
KSUB = 512                          # matmul free-dim granularity

Q = 127                             # Chebyshev nodes
QA = 128                            # padded quadratic-form size

# Schraudolph bf16 exp constants: int16 bits = max(A16*t + C16, 0) give a
# bf16 value ~ exp(t) with a stable multiplicative bias corrected by KAPPA.
A16 = 128.0 / float(np.log(2.0))
C16 = 16218.0
KAPPA = 1.1806

# pcol column indices
_C_W10, _C_W11, _C_B10, _C_B11 = 0, 1, 2, 3
_C_W200, _C_W201, _C_W210, _C_W211 = 4, 5, 6, 7
_C_B20, _C_B21, _C_W30, _C_W31, _C_B3 = 8, 9, 10, 11, 12
PCOLS = 13

# KDE chunking: 8 chunks of 1024 per tile through a 3-deep PSUM ring so the
# PE refill latency stays hidden; ownership interleaved A D A A D A A D to
# keep both engines streaming at the ~5:3 throughput ratio.
KCHUNK = 1024
CHUNK_STARTS = list(range(0, K_COMP, KCHUNK))
CHUNK_SIZES = [KCHUNK] * len(CHUNK_STARTS)


def act_chunks(t):
    return (0, 1, 2, 3, 4, 5, 6) if t == TILES - 1 else (0, 2, 3, 5, 6)


def dve_chunks(t):
    return (7,) if t == TILES - 1 else (1, 4, 7)


def _col_maps():
    amap, dmap = {}, {}
    ac = dc = 0
    for t in range(TILES):
        for ch in act_chunks(t):
            amap[(t, ch)] = ac
            ac += 1
        for ch in dve_chunks(t):
            dmap[(t, ch)] = dc
            dc += 1
    return amap, dmap, ac, dc


ACT_COL, DVE_COL, N_ACT_COLS, N_DVE_COLS = _col_maps()

_PROG = None
LAST_EXEC_NS = None


def build_program():
    import concourse.bass as bass
    import concourse.tile as tile
    from concourse import bacc, mybir
    from concourse.bass_isa import ReduceOp

    f32 = mybir.dt.float32
    f32r = mybir.dt.float32r
    bf16 = mybir.dt.bfloat16
    i16 = mybir.dt.int16
    Alu = mybir.AluOpType
    Act = mybir.ActivationFunctionType

    nc = bacc.Bacc("TRN2", target_bir_lowering=False, debug=False,
                   num_devices=N_CORES)

    empT_d = nc.declare_dram_parameter("empT", [16, K_COMP], f32r, isOutput=False)
    wT_d = nc.declare_dram_parameter("wT", [16, N_LOC], f32r, isOutput=False)
    pcol_d = nc.declare_dram_parameter("pcol", [P, TILES * PCOLS], f32, isOutput=False)
    nodes_d = nc.declare_dram_parameter("nodes", [P, Q], bf16, isOutput=False)
    gmat_d = nc.declare_dram_parameter("gmat", [QA, QA], bf16, isOutput=False)
    rvec_d = nc.declare_dram_parameter("rvec", [1, QA], bf16, isOutput=False)
    qact_d = nc.declare_dram_parameter("qact", [P, N_ACT_COLS], f32, isOutput=True)
    qdve_d = nc.declare_dram_parameter("qdve", [P, N_DVE_COLS], f32, isOutput=True)
    ssq_d = nc.declare_dram_parameter("ssq", [TILES, P], f32, isOutput=True)

    exp_scale = float(1.0 / A16)
    exp_bias = float(-C16 / A16)

    with tile.TileContext(nc) as tc:
        with (
            tc.tile_pool(name="const", bufs=1) as cpool,
            tc.tile_pool(name="i16p", bufs=3) as ipool,
            tc.tile_pool(name="mlpa", bufs=3) as mpool,
            tc.tile_pool(name="mlpb", bufs=3) as m2pool,
            tc.tile_pool(name="psum", bufs=3, space=bass.MemorySpace.PSUM) as ppool,
            tc.tile_pool(name="psum1", bufs=2, space=bass.MemorySpace.PSUM) as p1pool,
        ):
            # ---- constants / inputs (first pieces unblock tile-0 work) ----
            empT = cpool.tile([16, K_COMP], f32r)
            wT = cpool.tile([16, N_LOC], f32r)
            pcall = cpool.tile([P, TILES * PCOLS], f32)
            nodes = cpool.tile([P, Q], bf16)
            nc.sync.dma_start(empT[:, 0:2048], empT_d[:, 0:2048])
            nc.sync.dma_start(wT[:], wT_d[:])
            nc.sync.dma_start(pcall[:], pcol_d[:])
            nc.sync.dma_start(nodes[:], nodes_d[:])
            # bulk pieces go through SWDGE (gpsimd) to keep the serialized
            # HWDGE stage free for the latency-critical first pieces
            nc.gpsimd.dma_start(empT[:, 2048:5120], empT_d[:, 2048:5120])
            nc.gpsimd.dma_start(empT[:, 5120:K_COMP], empT_d[:, 5120:K_COMP])
            pcs = [pcall[:, t * PCOLS:(t + 1) * PCOLS] for t in range(TILES)]

            warm = cpool.tile([P, 1], f32)
            nc.vector.memset(warm[:], 0.0)
            nc.scalar.activation(warm[:], warm[:], Act.Exp)
            ebias = cpool.tile([P, 1], f32)
            nc.vector.memset(ebias[:], exp_bias)
            gmat = cpool.tile([QA, QA], bf16)
            nc.sync.dma_start(gmat[:], gmat_d[:])
            rvec = cpool.tile([1, QA], bf16)
            nc.sync.dma_start(rvec[:], rvec_d[:])
            ones_r = cpool.tile([1, QA], bf16)
            nc.vector.memset(ones_r[:], 1.0)
            ones_c = cpool.tile([P, 1], bf16)
            nc.vector.memset(ones_c[:], 1.0)

            qact_sb = cpool.tile([P, N_ACT_COLS], f32)
            qdve_sb = cpool.tile([P, N_DVE_COLS], f32)

            def emit_mlp(t):
                pc = pcs[t]
                # layer 1: per-node affine on nodes, then one tanh
                harg = mpool.tile([P, 2 * Q], bf16, tag="harg")
                for i in range(2):
                    nc.vector.tensor_scalar(
                        harg[:, i * Q:(i + 1) * Q], nodes[:],
                        pc[:, _C_W10 + i:_C_W10 + i + 1],
                        pc[:, _C_B10 + i:_C_B10 + i + 1],
                        Alu.mult, Alu.add)
                h = mpool.tile([P, 2 * Q], bf16, tag="h")
                nc.scalar.activation(h[:], harg[:], Act.Tanh)
                h0 = h[:, :Q]
                h1 = h[:, Q:]
                # layer 2
                garg = mpool.tile([P, 2 * Q], bf16, tag="garg")
                for i in range(2):
                    ti = m2pool.tile([P, Q], bf16, tag="ti")
                    nc.vector.tensor_scalar(
                        ti[:], h1,
                        pc[:, _C_W201 + 2 * i:_C_W201 + 2 * i + 1],
                        pc[:, _C_B20 + i:_C_B20 + i + 1],
                        Alu.mult, Alu.add)
                    nc.vector.scalar_tensor_tensor(
                        garg[:, i * Q:(i + 1) * Q], h0,
                        pc[:, _C_W200 + 2 * i:_C_W200 + 2 * i + 1],
                        ti[:], Alu.mult, Alu.add)
                g = mpool.tile([P, 2 * Q], bf16, tag="g")
                nc.scalar.activation(g[:], garg[:], Act.Tanh)
                # layer 3 -> Cs (incl b3), pad col 127 with zeros
                t3 = m2pool.tile([P, Q], bf16, tag="t3")
                nc.vector.tensor_scalar(
                    t3[:], g[:, :Q],
                    pc[:, _C_W30:_C_W30 + 1],
                    pc[:, _C_B3:_C_B3 + 1],
                    Alu.mult, Alu.add)
                cs = m2pool.tile([P, QA], bf16, tag="cs")
                nc.vector.scalar_tensor_tensor(
                    cs[:, :Q], g[:, Q:],
                    pc[:, _C_W31:_C_W31 + 1],
                    t3[:], Alu.mult, Alu.add)
                nc.vector.memset(cs[:, Q:QA], 0.0)
                cts = m2pool.tile([QA, P], bf16, tag="cts")
                nc.sync.dma_start_transpose(cts[:], cs[:])
                # quadratic form: ssq_n = cs_n^T G cs_n + r . cs_n
                mp = p1pool.tile([QA, P], f32, tag="mp")
                nc.tensor.matmul(mp[:], gmat[:], cts[:], start=True, stop=False)
                nc.tensor.matmul(mp[:], rvec[:], ones_r[:], start=False, stop=True)
                usq = m2pool.tile([QA, P], bf16, tag="usq")
                nc.vector.tensor_tensor(usq[:], cts[:], mp[:], Alu.mult)
                sred = m2pool.tile([QA, P], f32, tag="sred")
                nc.gpsimd.partition_all_reduce(sred[:], usq[:], P, ReduceOp.add)
                nc.sync.dma_start(ssq_d[t:t + 1, :], sred[0:1, :])

            def emit_kde(t):
                lhsT = wT[:, t * P:(t + 1) * P]
                achunks = act_chunks(t)
                for c, (k0, sz) in enumerate(zip(CHUNK_STARTS, CHUNK_SIZES)):
                    ps = ppool.tile([P, KCHUNK], f32, tag="ps",
                                    space=bass.MemorySpace.PSUM)
                    for s in range(sz // KSUB):
                        nc.tensor.matmul(
                            ps[:, s * KSUB:(s + 1) * KSUB],
                            lhsT,
                            empT[:, k0 + s * KSUB:k0 + (s + 1) * KSUB],
                            start=True, stop=True)
                    if c in achunks:
                        col = ACT_COL[(t, c)]
                        nc.scalar.activation(
                            ps[:, :sz], ps[:, :sz], Act.Exp,
                            bias=ebias[:], scale=exp_scale,
                            accum_out=qact_sb[:, col:col + 1])
                    else:
                        col = DVE_COL[(t, c)]
                        it = ipool.tile([P, KCHUNK], i16, tag="it")
                        nc.vector.tensor_scalar(
                            it[:, :sz], ps[:, :sz], 0.0, None, Alu.max)
                        bv = it[:, :sz].bitcast(bf16)
                        nc.vector.tensor_scalar(
                            bv, bv, 1.0, 0.0, Alu.mult, Alu.add,
                            accum_out=qdve_sb[:, col:col + 1])

            for t in range(TILES):
                emit_mlp(t)
                emit_kde(t)

            nc.sync.dma_start(qact_d[:], qact_sb[:])
            nc.gpsimd.dma_start(qdve_d[:], qdve_sb[:])

    nc.compile()
    return nc


def _get_prog():
    global _PROG
    if _PROG is None:
        _PROG = build_program()
    return _PROG


def host_prep(emp_samples, log_kde_rhos, x, y, eps, rand_idxs):
    """Returns (per-core in_maps, host-side combine context)."""
    emp = np.asarray(emp_samples, np.float32)
    logr = np.asarray(log_kde_rhos, np.float32)
    x = np.asarray(x, np.float64).reshape(-1)
    y = np.asarray(y, np.float64).reshape(-1)
    eps = np.asarray(eps, np.float32)
    idx = np.asarray(rand_idxs).astype(np.int64)

    # softplus in f32, matching jax.nn.softplus
    kde_std = np.logaddexp(np.float32(0.0), logr).astype(np.float32)
    kde_var = (kde_std * kde_std).astype(np.float32)

    esq = np.einsum("kd,kd->k", emp, emp, dtype=np.float32).astype(np.float32)
    colconst = (-0.5 * (D_W * LOG_2PI + D_W * np.log(kde_var))).astype(np.float32)
    a = (-0.5 / kde_var).astype(np.float32)

    A = np.float32(A16)
    empT = np.empty((16, K_COMP), np.float32)
    empT[:D_W] = (A * emp / kde_var[:, None]).T
    empT[D_W] = A * a
    empT[D_W + 1] = A * (colconst + a * esq)
    empT[D_W + 2] = 1.0

    std_g = kde_std[idx]
    w = (emp[idx] + eps * std_g[:, None]).astype(np.float32)
    wsq = np.einsum("nd,nd->n", w, w, dtype=np.float32).astype(np.float32)
    epssq = np.einsum("nd,nd->n", eps, eps, dtype=np.float32)
    m = (colconst[idx] - 0.5 * epssq).astype(np.float32)

    # Chebyshev grid on the x range and the quadratic form for
    # ssq = |Phi c - y|^2 (Phi: barycentric interpolation matrix).
    lo, hi = x.min(), x.max()
    kk = np.arange(Q)
    tch = np.cos(np.pi * kk / (Q - 1))[::-1]
    nodes = (lo + hi) / 2 + (hi - lo) / 2 * tch
    bw = np.ones(Q)
    bw[0] = bw[-1] = 0.5
    bw *= (-1.0) ** kk
    diff = x[:, None] - nodes[None, :]
    hit = np.abs(diff) < 1e-13
    with np.errstate(divide="ignore", invalid="ignore"):
        tmp = bw[None, :] / diff
        Phi = tmp / tmp.sum(1)[:, None]
    rows_hit = hit.any(1)
    Phi[rows_hit] = hit[rows_hit].astype(np.float64)

    G = np.zeros((QA, QA), np.float64)
    G[:Q, :Q] = Phi.T @ Phi
    r2 = np.zeros((1, QA), np.float64)
    r2[0, :Q] = -2.0 * (Phi.T @ y)
    sy2 = float((y * y).sum())

    gmat = G.astype(ml_bf16)
    rvec = r2.astype(ml_bf16)
    nodes_b = np.ascontiguousarray(
        np.broadcast_to(nodes.astype(ml_bf16), (P, Q)))

    in_maps = []
    for c in range(N_CORES):
        sl = slice(c * N_LOC, (c + 1) * N_LOC)
        wTc = np.empty((16, N_LOC), np.float32)
        wTc[:D_W] = w[sl].T
        wTc[D_W] = wsq[sl]
        wTc[D_W + 1] = 1.0
        wTc[D_W + 2] = np.float32(C16) - A * m[sl]
        # pcol packed partition-major: [128, TILES*13]
        pcp = np.ascontiguousarray(
            w[sl].reshape(TILES, P, PCOLS).transpose(1, 0, 2).reshape(P, TILES * PCOLS))
        in_maps.append({
            "empT": np.ascontiguousarray(empT),
            "wT": np.ascontiguousarray(wTc),
            "pcol": pcp,
            "nodes": nodes_b,
            "gmat": gmat,
            "rvec": rvec,
        })

    ctx = {"wsq": wsq, "m": m, "sy2": sy2}
    return in_maps, ctx


def host_combine(ctx, qsum, ssq_dev):
    m = ctx["m"].astype(np.float64)
    wsq = ctx["wsq"].astype(np.float64)

    q_lp = m + np.log(np.maximum(qsum, 1e-300)) - np.log(float(K_COMP))
    prior_lp = -0.5 * ALPHA * wsq + D_W * 0.5 * (np.log(ALPHA) - LOG_2PI)
    kl_term = (q_lp - prior_lp).mean()

    ssq = ssq_dev + ctx["sy2"]
    data_lp = (-0.5 * BETA) * ssq.mean() + B_X * 0.5 * (np.log(BETA) - LOG_2PI)
    return np.float32(data_lp - KL_BETA * kl_term)


def kernel(emp_samples, log_kde_rhos, x, y, eps, rand_idxs):
    global LAST_EXEC_NS
    from concourse.bass_utils import run_bass_kernel_spmd

    nc = _get_prog()
    in_maps, ctx = host_prep(emp_samples, log_kde_rhos, x, y, eps, rand_idxs)

    trace = bool(int(os.environ.get("BNN_TRACE", "0")))
    try:
        res = run_bass_kernel_spmd(nc, in_maps, core_ids=list(range(N_CORES)),
                                   trace=trace)
    except ModuleNotFoundError:
        res = run_bass_kernel_spmd(nc, in_maps, core_ids=list(range(N_CORES)))
    LAST_EXEC_NS = res.exec_time_ns

    qsum_parts = []
    ssq_parts = []
    for r in res.results:
        qa = r["qact"].astype(np.float64)
        qd = r["qdve"].astype(np.float64)
        qsum_loc = np.empty(N_LOC, np.float64)
        for t in range(TILES):
            tot = np.zeros(P, np.float64)
            for ch in act_chunks(t):
                tot += qa[:, ACT_COL[(t, ch)]]
            dtot = np.zeros(P, np.float64)
            for ch in dve_chunks(t):
                dtot += qd[:, DVE_COL[(t, ch)]]
            qsum_loc[t * P:(t + 1) * P] = tot + KAPPA * dtot
        qsum_parts.append(qsum_loc)
        ssq_parts.append(r["ssq"].astype(np.float64).reshape(N_LOC))

    qsum = np.concatenate(qsum_parts)
    ssq_dev = np.concatenate(ssq_parts)
    return host_combine(ctx, qsum, ssq_dev)
